# revision 3
# baseline (speedup 1.0000x reference)
"""Masked multi-head attention (CLS-token sparse attention) on 8 Trainium2
NeuronCores, data-parallel over batch (1 batch element per core).

Per-core math (all in transposed layouts to keep matmul operands natural):
  x^T [c, n], weights pre-transposed+scaled on host.
  q^T = wq_s^T-matmul, k^T, v natural [n, o] for the AV step.
  S^T[j, i] = k_h^T.T @ q_h^T  (per head, K=64 contraction on partitions)
  E = exp(S^T + maskbias[j])   (ACT, per-partition bias = key mask)
  [O'^T ; denom] = [v_h | 1].T @ E   (M=65: head dim + denominator row)
  out_attn^T = O'^T * (1/denom) broadcast  (GPSIMD partition_broadcast + DVE)
  y^T = wproj^T.T @ out_attn^T + bproj    (bias via DVE per-partition add)

All matmul inputs bf16, PSUM fp32, softmax pipeline fp32.
"""

import numpy as np
import ml_dtypes

B, N, C, H, D = 8, 1024, 1024, 16, 64
P = 128
KC = C // P      # 8 contraction chunks
OC = C // P      # 8 output-channel chunks
NB = N // 512    # 2 free-dim chunks of 512
JC = N // P      # 8 key chunks
NCORES = 8

_CACHE = {}


def _build_nc():
    import concourse.bass as bass
    import concourse.tile as tile
    from concourse import bacc, mybir

    bf16 = mybir.dt.bfloat16
    f32 = mybir.dt.float32

    nc = bacc.Bacc("TRN2", target_bir_lowering=False, debug=False)

    xt_d = nc.dram_tensor("xt", [C, N], bf16, kind="ExternalInput").ap()
    wqt_d = nc.dram_tensor("wqt", [C, C], bf16, kind="ExternalInput").ap()
    wkt_d = nc.dram_tensor("wkt", [C, C], bf16, kind="ExternalInput").ap()
    wvt_d = nc.dram_tensor("wvt", [C, C], bf16, kind="ExternalInput").ap()
    wpt_d = nc.dram_tensor("wpt", [C, C], bf16, kind="ExternalInput").ap()
    mb_d = nc.dram_tensor("mb", [N], f32, kind="ExternalInput").ap()
    bb_d = nc.dram_tensor("bb", [C], f32, kind="ExternalInput").ap()
    yt_d = nc.dram_tensor("yt", [C, N], f32, kind="ExternalOutput").ap()

    with tile.TileContext(nc) as tc:
        _kernel_body(nc, tc, mybir, xt_d, wqt_d, wkt_d, wvt_d, wpt_d,
                     mb_d, bb_d, yt_d)
    nc.compile()
    return nc


def _kernel_body(nc, tc, mybir, xt_d, wqt_d, wkt_d, wvt_d, wpt_d, mb_d, bb_d, yt_d):
    from contextlib import ExitStack
    bf16 = mybir.dt.bfloat16
    f32 = mybir.dt.float32
    Exp = mybir.ActivationFunctionType.Exp

    with ExitStack() as ctx:
        const = ctx.enter_context(tc.tile_pool(name="const", bufs=1))
        e_pool = ctx.enter_context(tc.tile_pool(name="e", bufs=6))
        r_pool = ctx.enter_context(tc.tile_pool(name="recip", bufs=4))
        bc_pool = ctx.enter_context(tc.tile_pool(name="bcast", bufs=4))
        y_pool = ctx.enter_context(tc.tile_pool(name="yt", bufs=3))
        pj_ps = ctx.enter_context(tc.tile_pool(name="pj_ps", bufs=3, space="PSUM"))
        sc_ps = ctx.enter_context(tc.tile_pool(name="sc_ps", bufs=2, space="PSUM"))
        av_ps = ctx.enter_context(tc.tile_pool(name="av_ps", bufs=2, space="PSUM"))

        # ---- resident inputs -------------------------------------------------
        xt = const.tile([P, KC, N], bf16)      # x^T   [p, kc, n]
        wqt = const.tile([P, KC, C], bf16)     # wq^T  [p, kc, o]  (pre-scaled)
        wkt = const.tile([P, KC, C], bf16)
        wvt = const.tile([P, KC, C], bf16)
        wpt = const.tile([P, KC, C], bf16)
        mb = const.tile([P, JC], f32)          # mask bias per key j
        bb = const.tile([P, OC], f32)          # proj bias per out channel o
        qt = const.tile([P, OC, N], bf16)      # q^T [p(o), oc, n]
        kt = const.tile([P, OC, N], bf16)
        vh = const.tile([P, JC, 65 * H], bf16)  # [p(n), jc, 65h+dd], col 64: ones
        oa = const.tile([P, KC, N], bf16)      # out_attn^T [p(c), cc, n]

        nc.gpsimd.dma_start(out=xt, in_=xt_d.rearrange("(k p) n -> p k n", p=P))
        nc.gpsimd.dma_start(out=wvt, in_=wvt_d.rearrange("(k p) o -> p k o", p=P))
        nc.gpsimd.dma_start(out=wqt, in_=wqt_d.rearrange("(k p) o -> p k o", p=P))
        nc.gpsimd.dma_start(out=wkt, in_=wkt_d.rearrange("(k p) o -> p k o", p=P))
        nc.gpsimd.dma_start(out=wpt, in_=wpt_d.rearrange("(k p) o -> p k o", p=P))
        nc.gpsimd.dma_start(out=mb, in_=mb_d.rearrange("(k p) -> p k", p=P))
        nc.gpsimd.dma_start(out=bb, in_=bb_d.rearrange("(k p) -> p k", p=P))

        # ones columns of vh (denominator trick), one strided memset per jc
        vh_r = vh.rearrange("p j (h e) -> p j h e", e=65)
        for jc in range(JC):
            nc.vector.memset(vh_r[:, jc, :, 64], 1.0)

        # ---- V projection: v[n, o] natural layout ---------------------------
        # lhsT = x^T chunk (M = n window), rhs = wv^T (N = o window)
        for nb2 in range(2):             # o halves (heads 8*nb2 .. 8*nb2+7)
            for mc in range(JC):         # n chunks
                ps = pj_ps.tile([P, 512], f32)
                for kc in range(KC):
                    nc.tensor.matmul(
                        ps, xt[:, kc, mc * P:(mc + 1) * P],
                        wvt[:, kc, nb2 * 512:(nb2 + 1) * 512],
                        start=(kc == 0), stop=(kc == KC - 1))
                for hh in range(8):
                    h = nb2 * 8 + hh
                    nc.vector.tensor_copy(
                        vh[:, mc, 65 * h:65 * h + 64],
                        ps[:, hh * 64:(hh + 1) * 64])

        # ---- per head-pair: Q/K projection chunk then attention -------------
        for g in range(OC):
            ha, hb = 2 * g, 2 * g + 1
            # q^T / k^T output chunk oc=g  (M = o window g)
            for (wt, dst) in ((wqt, qt), (wkt, kt)):
                for nb2 in range(NB):
                    ps = pj_ps.tile([P, 512], f32)
                    for kc in range(KC):
                        nc.tensor.matmul(
                            ps, wt[:, kc, g * P:(g + 1) * P],
                            xt[:, kc, nb2 * 512:(nb2 + 1) * 512],
                            start=(kc == 0), stop=(kc == KC - 1))
                    nc.vector.tensor_copy(dst[:, g, nb2 * 512:(nb2 + 1) * 512], ps)

            # attention for heads ha (partitions 0:64) and hb (64:128)
            for ic in range(NB):
                i0 = ic * 512
                avs = {}
                for h, p0 in ((ha, 0), (hb, 64)):
                    avs[h] = av_ps.tile([65, 512], f32, name=f"av_{h}_{ic}", tag="av")
                for jc in range(JC):
                    es = {}
                    for h, p0 in ((ha, 0), (hb, 64)):
                        s_ps = sc_ps.tile([P, 512], f32)
                        nc.tensor.matmul(
                            s_ps,
                            kt[p0:p0 + 64, g, jc * P:(jc + 1) * P],
                            qt[p0:p0 + 64, g, i0:i0 + 512],
                            start=True, stop=True)
                        e = e_pool.tile([P, 512], bf16, name=f"e_{h}_{ic}_{jc}",
                                        tag="e")
                        nc.scalar.activation(e, s_ps, Exp, bias=mb[:, jc:jc + 1])
                        es[h] = e
                    for h in (ha, hb):
                        nc.tensor.matmul(
                            avs[h], vh[:, jc, 65 * h:65 * h + 65], es[h],
                            start=(jc == 0), stop=(jc == JC - 1))
                for h, p0 in ((ha, 0), (hb, 64)):
                    recip = r_pool.tile([1, 512], f32)
                    nc.vector.reciprocal(recip, avs[h][64:65, :])
                    bc = bc_pool.tile([64, 512], f32)
                    nc.gpsimd.partition_broadcast(bc, recip)
                    nc.vector.tensor_mul(
                        oa[p0:p0 + 64, g, i0:i0 + 512], avs[h][0:64, :], bc)

        # ---- output projection: y^T[o, n] + bias ----------------------------
        for oc in range(OC):
            for nb2 in range(NB):
                ps = pj_ps.tile([P, 512], f32)
                for kc in range(KC):
                    nc.tensor.matmul(
                        ps, wpt[:, kc, oc * P:(oc + 1) * P],
                        oa[:, kc, nb2 * 512:(nb2 + 1) * 512],
                        start=(kc == 0), stop=(kc == KC - 1))
                yt = y_pool.tile([P, 512], f32)
                nc.vector.tensor_scalar_add(yt, ps, bb[:, oc:oc + 1])
                nc.gpsimd.dma_start(
                    out=yt_d[oc * P:(oc + 1) * P, nb2 * 512:(nb2 + 1) * 512],
                    in_=yt)


def _prep_inputs(x, mask, wq, wk, wv, wproj, bproj):
    """Host-side preprocessing: transposes, scaling, dtype casts."""
    bf = ml_dtypes.bfloat16
    scale = D ** (-0.5)
    wqt = np.ascontiguousarray((wq * scale).T).astype(bf)
    wkt = np.ascontiguousarray(wk.T).astype(bf)
    wvt = np.ascontiguousarray(wv.T).astype(bf)
    wpt = np.ascontiguousarray(wproj.T).astype(bf)
    full_mask = np.concatenate(
        [np.ones((B, 1), dtype=bool), np.asarray(mask)], axis=1)
    mb = np.where(full_mask, np.float32(0.0), np.float32(-1e9))
    bb = np.ascontiguousarray(bproj.astype(np.float32))
    in_maps = []
    for b in range(B):
        xt = np.ascontiguousarray(np.asarray(x[b]).T).astype(bf)
        in_maps.append({
            "xt": xt, "wqt": wqt, "wkt": wkt, "wvt": wvt, "wpt": wpt,
            "mb": np.ascontiguousarray(mb[b]), "bb": bb,
        })
    return in_maps


def get_nc():
    if "nc" not in _CACHE:
        _CACHE["nc"] = _build_nc()
    return _CACHE["nc"]


def kernel(x, mask, wq, wk, wv, wproj, bproj):
    from concourse.bass_utils import run_bass_kernel_spmd
    nc = get_nc()
    in_maps = _prep_inputs(x, mask, wq, wk, wv, wproj, bproj)
    res = run_bass_kernel_spmd(nc, in_maps, core_ids=list(range(NCORES)))
    out = np.empty((B, N, C), np.float32)
    for b in range(B):
        out[b] = res.results[b]["yt"].T
    return out


if __name__ == "__main__":
    rng = np.random.default_rng(0)
    ins = {
        "x": rng.standard_normal((B, N, C), dtype=np.float32),
        "mask": rng.integers(0, 2, (B, N - 1)).astype(bool),
        "wq": rng.standard_normal((C, C), dtype=np.float32) * 0.02,
        "wk": rng.standard_normal((C, C), dtype=np.float32) * 0.02,
        "wv": rng.standard_normal((C, C), dtype=np.float32) * 0.02,
        "wproj": rng.standard_normal((C, C), dtype=np.float32) * 0.02,
        "bproj": rng.standard_normal((C,), dtype=np.float32) * 0.02,
    }
    o = kernel(**ins)
    print(o.shape, o.dtype)


# revision 5
# speedup vs baseline: 155.4681x; 155.4681x over previous
"""Masked multi-head attention (CLS-token sparse attention) on 8 Trainium2
NeuronCores, data-parallel over batch (1 batch element per core).

Per-core math (all in transposed layouts to keep matmul operands natural):
  x^T [c, n], weights pre-transposed+scaled on host.
  q^T = wq_s^T-matmul, k^T, v natural [n, o] for the AV step.
  S^T[j, i] = k_h^T.T @ q_h^T  (per head, K=64 contraction on partitions)
  E = exp(S^T + maskbias[j])   (ACT, per-partition bias = key mask)
  [O'^T ; denom] = [v_h | 1].T @ E   (M=65: head dim + denominator row)
  out_attn^T = O'^T * (1/denom) broadcast  (GPSIMD partition_broadcast + DVE)
  y^T = wproj^T.T @ out_attn^T + bproj    (bias via DVE per-partition add)

All matmul inputs bf16, PSUM fp32, softmax pipeline fp32.
"""

import numpy as np
import ml_dtypes

B, N, C, H, D = 8, 1024, 1024, 16, 64
P = 128
KC = C // P      # 8 contraction chunks
OC = C // P      # 8 output-channel chunks
NB = N // 512    # 2 free-dim chunks of 512
JC = N // P      # 8 key chunks
NCORES = 8

_CACHE = {}


def _build_nc(repeat=1):
    import concourse.bass as bass
    import concourse.tile as tile
    from concourse import bacc, mybir
    from contextlib import nullcontext

    bf16 = mybir.dt.bfloat16
    f32 = mybir.dt.float32

    nc = bacc.Bacc("TRN2", target_bir_lowering=False, debug=False)

    xt_d = nc.dram_tensor("xt", [C, N], bf16, kind="ExternalInput").ap()
    wqt_d = nc.dram_tensor("wqt", [C, C], bf16, kind="ExternalInput").ap()
    wkt_d = nc.dram_tensor("wkt", [C, C], bf16, kind="ExternalInput").ap()
    wvt_d = nc.dram_tensor("wvt", [C, C], bf16, kind="ExternalInput").ap()
    wpt_d = nc.dram_tensor("wpt", [C, C], bf16, kind="ExternalInput").ap()
    mb_d = nc.dram_tensor("mb", [N], f32, kind="ExternalInput").ap()
    bb_d = nc.dram_tensor("bb", [C], f32, kind="ExternalInput").ap()
    yt_d = nc.dram_tensor("yt", [C, N], f32, kind="ExternalOutput").ap()

    with tile.TileContext(nc) as tc:
        loop = tc.For_i(0, repeat, 1) if repeat > 1 else nullcontext()
        with loop:
            _kernel_body(nc, tc, mybir, xt_d, wqt_d, wkt_d, wvt_d, wpt_d,
                         mb_d, bb_d, yt_d)
    nc.compile()
    return nc


def _kernel_body(nc, tc, mybir, xt_d, wqt_d, wkt_d, wvt_d, wpt_d, mb_d, bb_d, yt_d):
    from contextlib import ExitStack
    bf16 = mybir.dt.bfloat16
    f32 = mybir.dt.float32
    Exp = mybir.ActivationFunctionType.Exp

    with ExitStack() as ctx:
        const = ctx.enter_context(tc.tile_pool(name="const", bufs=1))
        e_pool = ctx.enter_context(tc.tile_pool(name="e", bufs=6))
        r_pool = ctx.enter_context(tc.tile_pool(name="recip", bufs=4))
        bc_pool = ctx.enter_context(tc.tile_pool(name="bcast", bufs=4))
        y_pool = ctx.enter_context(tc.tile_pool(name="yt", bufs=3))
        pj_ps = ctx.enter_context(tc.tile_pool(name="pj_ps", bufs=3, space="PSUM"))
        sc_ps = ctx.enter_context(tc.tile_pool(name="sc_ps", bufs=2, space="PSUM"))
        av_ps = ctx.enter_context(tc.tile_pool(name="av_ps", bufs=2, space="PSUM"))

        # ---- resident inputs -------------------------------------------------
        xt = const.tile([P, KC, N], bf16)      # x^T   [p, kc, n]
        wqt = const.tile([P, KC, C], bf16)     # wq^T  [p, kc, o]  (pre-scaled)
        wkt = const.tile([P, KC, C], bf16)
        wvt = const.tile([P, KC, C], bf16)
        wpt = const.tile([P, KC, C], bf16)
        mb = const.tile([P, JC], f32)          # mask bias per key j
        bb = const.tile([P, OC], f32)          # proj bias per out channel o
        qt = const.tile([P, OC, N], bf16)      # q^T [p(o), oc, n]
        kt = const.tile([P, OC, N], bf16)
        vh = const.tile([P, JC, 65 * H], bf16)  # [p(n), jc, 65h+dd], col 64: ones
        oa = const.tile([P, KC, N], bf16)      # out_attn^T [p(c), cc, n]

        nc.gpsimd.dma_start(out=xt, in_=xt_d.rearrange("(k p) n -> p k n", p=P))
        nc.gpsimd.dma_start(out=wvt, in_=wvt_d.rearrange("(k p) o -> p k o", p=P))
        nc.gpsimd.dma_start(out=wqt, in_=wqt_d.rearrange("(k p) o -> p k o", p=P))
        nc.gpsimd.dma_start(out=wkt, in_=wkt_d.rearrange("(k p) o -> p k o", p=P))
        nc.gpsimd.dma_start(out=wpt, in_=wpt_d.rearrange("(k p) o -> p k o", p=P))
        nc.gpsimd.dma_start(out=mb, in_=mb_d.rearrange("(k p) -> p k", p=P))
        nc.gpsimd.dma_start(out=bb, in_=bb_d.rearrange("(k p) -> p k", p=P))

        # ones columns of vh (denominator trick), one strided memset per jc
        vh_r = vh.rearrange("p j (h e) -> p j h e", e=65)
        for jc in range(JC):
            nc.vector.memset(vh_r[:, jc, :, 64], 1.0)

        # ---- V projection: v[n, o] natural layout ---------------------------
        # lhsT = x^T chunk (M = n window), rhs = wv^T (N = o window)
        for nb2 in range(2):             # o halves (heads 8*nb2 .. 8*nb2+7)
            for mc in range(JC):         # n chunks
                ps = pj_ps.tile([P, 512], f32)
                for kc in range(KC):
                    nc.tensor.matmul(
                        ps, xt[:, kc, mc * P:(mc + 1) * P],
                        wvt[:, kc, nb2 * 512:(nb2 + 1) * 512],
                        start=(kc == 0), stop=(kc == KC - 1))
                for hh in range(8):
                    h = nb2 * 8 + hh
                    nc.vector.tensor_copy(
                        vh[:, mc, 65 * h:65 * h + 64],
                        ps[:, hh * 64:(hh + 1) * 64])

        # ---- per head-pair: Q/K projection chunk then attention -------------
        for g in range(OC):
            ha, hb = 2 * g, 2 * g + 1
            # q^T / k^T output chunk oc=g  (M = o window g)
            for (wt, dst) in ((wqt, qt), (wkt, kt)):
                for nb2 in range(NB):
                    ps = pj_ps.tile([P, 512], f32)
                    for kc in range(KC):
                        nc.tensor.matmul(
                            ps, wt[:, kc, g * P:(g + 1) * P],
                            xt[:, kc, nb2 * 512:(nb2 + 1) * 512],
                            start=(kc == 0), stop=(kc == KC - 1))
                    nc.vector.tensor_copy(dst[:, g, nb2 * 512:(nb2 + 1) * 512], ps)

            # attention for heads ha (partitions 0:64) and hb (64:128)
            for ic in range(NB):
                i0 = ic * 512
                avs = {}
                for h, p0 in ((ha, 0), (hb, 64)):
                    avs[h] = av_ps.tile([65, 512], f32, name=f"av_{h}_{ic}", tag="av")
                for jc in range(JC):
                    es = {}
                    for h, p0 in ((ha, 0), (hb, 64)):
                        s_ps = sc_ps.tile([P, 512], f32)
                        nc.tensor.matmul(
                            s_ps,
                            kt[p0:p0 + 64, g, jc * P:(jc + 1) * P],
                            qt[p0:p0 + 64, g, i0:i0 + 512],
                            start=True, stop=True)
                        e = e_pool.tile([P, 512], bf16, name=f"e_{h}_{ic}_{jc}",
                                        tag="e")
                        nc.scalar.activation(e, s_ps, Exp, bias=mb[:, jc:jc + 1])
                        es[h] = e
                    for h in (ha, hb):
                        nc.tensor.matmul(
                            avs[h], vh[:, jc, 65 * h:65 * h + 65], es[h],
                            start=(jc == 0), stop=(jc == JC - 1))
                for h, p0 in ((ha, 0), (hb, 64)):
                    recip = r_pool.tile([1, 512], f32)
                    nc.vector.reciprocal(recip, avs[h][64:65, :])
                    bc = bc_pool.tile([64, 512], f32)
                    nc.gpsimd.partition_broadcast(bc, recip)
                    nc.vector.tensor_mul(
                        oa[p0:p0 + 64, g, i0:i0 + 512], avs[h][0:64, :], bc)

        # ---- output projection: y^T[o, n] + bias ----------------------------
        for oc in range(OC):
            for nb2 in range(NB):
                ps = pj_ps.tile([P, 512], f32)
                for kc in range(KC):
                    nc.tensor.matmul(
                        ps, wpt[:, kc, oc * P:(oc + 1) * P],
                        oa[:, kc, nb2 * 512:(nb2 + 1) * 512],
                        start=(kc == 0), stop=(kc == KC - 1))
                yt = y_pool.tile([P, 512], f32)
                nc.vector.tensor_scalar_add(yt, ps, bb[:, oc:oc + 1])
                nc.gpsimd.dma_start(
                    out=yt_d[oc * P:(oc + 1) * P, nb2 * 512:(nb2 + 1) * 512],
                    in_=yt)


def _prep_inputs(x, mask, wq, wk, wv, wproj, bproj):
    """Host-side preprocessing: transposes, scaling, dtype casts."""
    bf = ml_dtypes.bfloat16
    scale = D ** (-0.5)
    wqt = np.ascontiguousarray((wq * scale).T).astype(bf)
    wkt = np.ascontiguousarray(wk.T).astype(bf)
    wvt = np.ascontiguousarray(wv.T).astype(bf)
    wpt = np.ascontiguousarray(wproj.T).astype(bf)
    full_mask = np.concatenate(
        [np.ones((B, 1), dtype=bool), np.asarray(mask)], axis=1)
    mb = np.where(full_mask, np.float32(0.0), np.float32(-1e9))
    bb = np.ascontiguousarray(bproj.astype(np.float32))
    in_maps = []
    for b in range(B):
        xt = np.ascontiguousarray(np.asarray(x[b]).T).astype(bf)
        in_maps.append({
            "xt": xt, "wqt": wqt, "wkt": wkt, "wvt": wvt, "wpt": wpt,
            "mb": np.ascontiguousarray(mb[b]), "bb": bb,
        })
    return in_maps


def get_nc(repeat=1):
    key = ("nc", repeat)
    if key not in _CACHE:
        _CACHE[key] = _build_nc(repeat)
    return _CACHE[key]


def kernel(x, mask, wq, wk, wv, wproj, bproj):
    from concourse.bass_utils import run_bass_kernel_spmd
    nc = get_nc()
    in_maps = _prep_inputs(x, mask, wq, wk, wv, wproj, bproj)
    res = run_bass_kernel_spmd(nc, in_maps, core_ids=list(range(NCORES)))
    out = np.empty((B, N, C), np.float32)
    for b in range(B):
        out[b] = res.results[b]["yt"].T
    return out


if __name__ == "__main__":
    rng = np.random.default_rng(0)
    ins = {
        "x": rng.standard_normal((B, N, C), dtype=np.float32),
        "mask": rng.integers(0, 2, (B, N - 1)).astype(bool),
        "wq": rng.standard_normal((C, C), dtype=np.float32) * 0.02,
        "wk": rng.standard_normal((C, C), dtype=np.float32) * 0.02,
        "wv": rng.standard_normal((C, C), dtype=np.float32) * 0.02,
        "wproj": rng.standard_normal((C, C), dtype=np.float32) * 0.02,
        "bproj": rng.standard_normal((C,), dtype=np.float32) * 0.02,
    }
    o = kernel(**ins)
    print(o.shape, o.dtype)
